# revision 49
# baseline (speedup 1.0000x reference)
"""Trainium2 Bass kernel for nn_BertSelfAttention_10110353015430.

Sharding: Megatron-style tensor parallel over heads. 16 heads / 8 cores =
2 heads per core. Each core computes the QKV projection for its 2 heads
(both batches), RoPE, full attention for its heads, and a partial
o-projection (its 128 columns of the 1024-wide contraction). The host
sums the 8 fp16 partials in f64.

v2 design (from the v1 HW trace: attention ran at PE half-clock with
fp32r and LDWEIGHTS un-overlapped; ACT exp is the true floor at ~110us):
  - All matmul operands fp16 (tolerance 2e-2; fp16 keeps ~5e-4): FWL
    weight loads, guaranteed 1 col/cycle streaming, half DMA traffic,
    2x DVE rate on 16-bit ops.
  - Scores for the two heads are row-packed: both heads' K^T tiles loaded
    to PE row groups 0-63 / 64-127 (tile_position auto-derived from
    base partitions), streaming concurrently -> ~512 cycles per pair.
  - Items are (batch, 512-token q-group, k-tile): one [128,1024] fp32
    PSUM scores tile (h0|h1 halves), one [128,1024] exp -> fp16 pt.
  - PV keeps the ones-column trick (V'=[V|1], M=65): denominator falls
    out of the PV matmul at psum row 64. ov psum [128,1024] packs both
    heads' [65,512] accumulators. PSUM = 2x s + 2x ov = exactly 8 banks.
  - softmax denominators: reciprocal_approx_fast (single custom DVE op,
    ~5x faster than reciprocal; 51 ULP is plenty for 2e-2).
  - RoPE fused out of PSUM per 512-chunk on DVE (no separate q/k psum
    copies on ACT): rot(x) = x*rc + swap_pairs(x)*rs, fp16 out.
  - o-projection runs as phase 3 after attention (PSUM is fully used
    during attention); out DMA'd as fp16 (host upcasts + sums).

Self-contained: hardcodes all shapes; no sibling imports, no file reads.
"""

import os
from contextlib import ExitStack

import numpy as np

import concourse.bass as bass
import concourse.mybir as mybir
import concourse.tile as tile
from concourse import bacc, bass_utils
from concourse.bass import ds, ts
from concourse.masks import make_identity

B, L, D = 2, 2048, 1024
H, HD = 16, 64
NCORES = 8
HLOC = H // NCORES          # 2 heads per core
NT = B * L                  # 4096 tokens, laid out [b0 | b1]
F32 = mybir.dt.float32

_MM_DT_NAME = os.environ.get("KERNEL_MM_DT", "float16")
MM_DT = getattr(mybir.dt, _MM_DT_NAME)
NP_MM_DT = np.float16 if _MM_DT_NAME == "float16" else np.dtype("bfloat16")

# packed-input layout (element offsets into the single fp16 input blob)
_PACK_SIZES = [D * NT, D * 384, 128 * D, 128 * L, 128 * L]
PACK_OFFS = []
_off = 0
for _s in _PACK_SIZES:
    PACK_OFFS.append(_off)
    _off += _s
PACK_OFFS.append(_off)
PACK_TOTAL = _off


def build_body(tc, ins, outs, dbg=None):
    """Per-core program. ins/outs: dicts of DRAM APs.

    ins (all fp16):
      xT    [1024, 4096]  X^T, tokens = [batch0(2048) | batch1(2048)]
      wqkvT [1024, 384]   cols: q-feats(128) | k-feats(128) | v-feats(128)
      woT   [128, 1024]   rows = this core's 128 attn cols, cols = out dim
      rc    [128, 2048]   RoPE cos table: rc[p, t] = cos[t % L, (p % 64)//2]
      rs    [128, 2048]   RoPE signed sin: -sin on even hd dims, +sin on odd
    outs:
      out   [4096, 1024]  fp16 partial o-projection
    """
    nc = tc.nc
    xT, wqkvT, woT = ins["xT"], ins["wqkvT"], ins["woT"]
    rc, rs = ins["rc"], ins["rs"]
    outp = outs["out"]
    swap_mask = [j + 1 if j % 2 == 0 else j - 1 for j in range(32)]

    with ExitStack() as ctx:
        sb = ctx.enter_context(tc.tile_pool(name="sb", bufs=1))
        xp = ctx.enter_context(tc.tile_pool(name="xp", bufs=2))
        rp = ctx.enter_context(tc.tile_pool(name="rp", bufs=3))

        # ---- persistent tiles ----
        w_sb = sb.tile([128, 8, 384], MM_DT, tag="w")
        wr = wqkvT.rearrange("(c p) f -> p c f", p=128)
        # split so the first chunk's matmuls wait on 96KB, not 768KB
        nc.gpsimd.dma_start(w_sb[:, 0:1, :], wr[:, 0:1, :])
        nc.gpsimd.dma_start(w_sb[:, 1:8, :], wr[:, 1:8, :])
        woT_sb = sb.tile([128, 1024], MM_DT, tag="wo")
        nc.gpsimd.dma_start(woT_sb, woT)
        rc_sb = sb.tile([128, L], MM_DT, tag="rc")
        nc.gpsimd.dma_start(rc_sb, rc)
        rs_sb = sb.tile([128, L], MM_DT, tag="rs")
        nc.gpsimd.dma_start(rs_sb, rs)

        ident = sb.tile([128, 128], F32, tag="id")
        make_identity(nc, ident)

        # Q^T | K^T fp16 post-RoPE, partitions = [h0 hd(64) | h1 hd(64)].
        # Split per batch so batch-0 attention doesn't dep-wait on batch-1
        # phase-1 writes (tile dep tracking is per-tile).
        qkt_b = [
            sb.tile([128, 2, L], MM_DT, tag=f"qkt{b}", name=f"qkt{b}")
            for b in range(B)
        ]
        # V per 128-token tile, per head, ones column at free idx 64
        vall_b = [
            sb.tile([128, 16, HLOC, 65], MM_DT, tag=f"vall{b}", name=f"vall{b}")
            for b in range(B)
        ]
        for vb in vall_b:
            nc.vector.memset(vb, 1.0)
        # attention out O^T (normalized), partitions = [h0(64) | h1(64)],
        # one tile per (batch, 512-token q-group) so phase-3 o-proj tiles
        # only dep on their own group's normalize.
        ot_g = [
            sb.tile([128, 512], MM_DT, tag=f"ot{g}", name=f"ot{g}")
            for g in range(8)
        ]

        # ---- phase 1: QKV projection + fused RoPE + V transpose ----
        with tc.tile_pool(name="pq", bufs=2, space="PSUM") as pq:
            for bb in range(B):
                for tch in range(4 * bb, 4 * bb + 4):   # 512-token chunks
                    sl = ds(tch * 512, 512)
                    csl = ds((tch % 4) * 512, 512)      # within-batch pos
                    q_ps = pq.tile([128, 512], F32, tag="q")
                    k_ps = pq.tile([128, 512], F32, tag="k")
                    v_ps = pq.tile([128, 512], F32, tag="v")
                    xr = xT.rearrange("(c p) t -> p c t", p=128)
                    xb8 = xp.tile([128, 8, 512], MM_DT, tag="xb")
                    if tch == 0:
                        # fine-grained first chunk: the first matmul only
                        # waits on a 128KB slice instead of 512KB
                        nc.sync.dma_start(xb8[:, 0:1, :], xr[:, 0:1, sl])
                        nc.sync.dma_start(xb8[:, 1:4, :], xr[:, 1:4, sl])
                    else:
                        nc.sync.dma_start(xb8[:, 0:4, :], xr[:, 0:4, sl])
                    nc.scalar.dma_start(xb8[:, 4:8, :], xr[:, 4:8, sl])
                    for dch in range(8):      # contraction chunks of 128
                        xb = xb8[:, dch, :]
                        st, sp = dch == 0, dch == 7
                        nc.tensor.matmul(
                            q_ps, w_sb[:, dch, 0:128], xb, start=st, stop=sp
                        )
                        nc.tensor.matmul(
                            k_ps, w_sb[:, dch, 128:256], xb, start=st, stop=sp
                        )
                        nc.tensor.matmul(
                            v_ps, w_sb[:, dch, 256:384], xb, start=st, stop=sp
                        )
                    # Drain all q/k/v psum via ACT copies so the psum pool
                    # frees promptly (a DVE-gated drain delays the phase-2
                    # pool open past the 3.4us HAM window -> cold attention).
                    vt0 = rp.tile([128, 512], F32, tag="vt0")
                    nc.scalar.copy(vt0, v_ps)
                    qk_st = rp.tile([128, 2, 512], MM_DT, tag="qk_st")
                    nc.scalar.copy(qk_st[:, 0, :], q_ps)
                    nc.scalar.copy(qk_st[:, 1, :], k_ps)
                    for i in range(4):
                        tt = (tch % 4) * 4 + i
                        vtp = pq.tile([128, 128], F32, tag="vt")
                        nc.tensor.transpose(
                            vtp, vt0[:, ds(i * 128, 128)], ident
                        )
                        # on ACT: keeps the psum pool drain off the DVE
                        # queue (DVE runs the RoPE backlog); single strided
                        # copy covers both heads' 64-col blocks
                        nc.scalar.copy(
                            vall_b[bb][:, tt, :, 0:64],
                            vtp.rearrange("p (h d) -> p h d", h=2),
                        )
                    # RoPE from fp16 staging: rot(x) = x*rc + swap(x)*rs
                    # (all-16-bit DVE ops run at 2x rate)
                    for si in range(2):
                        ps = qk_st[:, si, :]
                        yt = rp.tile([128, 512], MM_DT, tag="yt")
                        yt2 = rp.tile([128, 512], MM_DT, tag="yt2")
                        qko = qkt_b[bb][:, si, csl]
                        nc.vector.stream_shuffle(yt, ps, swap_mask)
                        nc.vector.tensor_mul(qko, ps, rc_sb[:, csl])
                        nc.vector.tensor_mul(yt2, yt, rs_sb[:, csl])
                        nc.vector.tensor_add(qko, qko, yt2)

        # ---- phase 2: attention ----
        ptp = ctx.enter_context(tc.tile_pool(name="ptp", bufs=6))
        dnp = ctx.enter_context(tc.tile_pool(name="dnp", bufs=3))
        obp = ctx.enter_context(tc.tile_pool(name="obp", bufs=4))
        # items: (batch, 512-token q-group, k-tile)
        items = [
            (b, qg, kt) for b in range(B) for qg in range(4) for kt in range(16)
        ]
        LAG = 3
        state = {}

        def emit_norm(b, qg, ov):
            if dbg is not None and b == 0 and qg == 0:
                dden = dnp.tile([1, 1024], F32, tag="dden")
                nc.vector.tensor_copy(dden, ov[ds(64, 1), :])
                nc.sync.dma_start(dbg["dbg_den"], dden)
            # head-interleaved so the two gpsimd broadcasts overlap DVE work
            recs, denbs = [], []
            for h in range(HLOC):
                den = dnp.tile([1, 512], F32, tag=f"den{h}", name="den")
                nc.vector.tensor_copy(den, ov[ds(64, 1), ds(h * 512, 512)])
                rec = dnp.tile([1, 512], F32, tag=f"rec{h}", name="rec")
                nc.vector.reciprocal_approx_fast(rec, den)
                recs.append(rec)
            for h in range(HLOC):
                denb = dnp.tile([64, 512], F32, tag=f"denb{h}", name="denb")
                nc.gpsimd.partition_broadcast(denb, recs[h])
                denbs.append(denb)
            for h in range(HLOC):
                nc.vector.tensor_mul(
                    ot_g[b * 4 + qg][ds(h * 64, 64), :],
                    ov[0:64, ds(h * 512, 512)],
                    denbs[h],
                )

        with tc.tile_pool(name="pa", bufs=2, space="PSUM") as pa:
            for i in range(len(items) + LAG):
                if i < len(items):
                    b, qg, kt = items[i]
                    s_ps = pa.tile([128, 1024], F32, tag="s")
                    for h in range(HLOC):
                        kth = qkt_b[b][ds(h * 64, 64), 1, ds(kt * 128, 128)]
                        qth = qkt_b[b][ds(h * 64, 64), 0, ds(qg * 512, 512)]
                        nc.tensor.matmul(
                            s_ps[:, ds(h * 512, 512)], kth, qth,
                            start=True, stop=True,
                        )
                    pt = ptp.tile([128, 1024], MM_DT, tag="pt")
                    nc.scalar.activation(
                        pt, s_ps, mybir.ActivationFunctionType.Exp,
                        scale=0.125,
                    )
                    state[i] = pt
                    if dbg is not None and i == 0:
                        nc.sync.dma_start(dbg["dbg_pt"], pt)
                if i >= LAG:
                    j = i - LAG
                    b2, qg2, kt2 = items[j]
                    if kt2 == 0:
                        ov = pa.tile([128, 1024], F32, tag="ov", name="ov")
                        state[(b2, qg2, "ov")] = ov
                    ov2 = state[(b2, qg2, "ov")]
                    pt_prev = state.pop(j)
                    for h in range(HLOC):
                        nc.tensor.matmul(
                            ov2[0:65, ds(h * 512, 512)],
                            vall_b[b2][:, kt2, h, :],
                            pt_prev[:, ds(h * 512, 512)],
                            start=(kt2 == 0),
                            stop=(kt2 == 15),
                        )
                    if kt2 == 15:
                        emit_norm(b2, qg2, ov2)
                        del state[(b2, qg2, "ov")]

            # ---- phase 3: o-projection over 32 token-tiles, emitted
            # inside the attention psum pool (alternating s/ov tags) so
            # there is no pool-transition barrier: early tiles start the
            # moment the tail items free their psum bufs, and only the
            # last group's 4 tiles wait on the final normalize.
            for tt in range(32):
                op_ps = pa.tile(
                    [128, 1024], F32, tag="s" if tt % 2 == 0 else "ov",
                    name="op_ps",
                )
                for ni in range(2):
                    nc.tensor.matmul(
                        op_ps[:, ds(ni * 512, 512)],
                        ot_g[tt // 4][:, ds((tt % 4) * 128, 128)],
                        woT_sb[:, ds(ni * 512, 512)],
                        start=True,
                        stop=True,
                    )
                # whole-tile copies alternating engines: amortizes the
                # ~300ns per-instruction overhead vs split halves
                ob = obp.tile([128, 1024], MM_DT, tag="ob")
                if tt % 2 == 0:
                    nc.vector.tensor_copy(ob, op_ps)
                else:
                    nc.scalar.copy(ob, op_ps)
                nc.sync.dma_start(outp[ds(tt * 128, 128), :], ob)

        if dbg is not None:
            for b in range(B):
                nc.sync.dma_start(
                    dbg["dbg_qkt"][:, ds(b * 2 * L, 2 * L)],
                    qkt_b[b].rearrange("p a t -> p (a t)"),
                )
                nc.sync.dma_start(
                    dbg["dbg_vall"][:, ds(b * 16 * HLOC * 65, 16 * HLOC * 65)],
                    vall_b[b].rearrange("p a h f -> p (a h f)"),
                )
            for g in range(8):
                nc.sync.dma_start(dbg["dbg_ot"][:, ds(g * 512, 512)], ot_g[g])


def _prep_inputs(hidden_states, w_qkv, w_o, freqs_cos, freqs_sin):
    """Host-side prep: transpose X, slice per-core weights, RoPE tables."""
    x = np.ascontiguousarray(
        np.asarray(hidden_states, dtype=np.float32).reshape(NT, D).T
    ).astype(NP_MM_DT)  # [1024, 4096]
    w_qkv = np.asarray(w_qkv, dtype=np.float32)
    w_o = np.asarray(w_o, dtype=np.float32)
    cosT = np.asarray(freqs_cos, dtype=np.float32).T     # [32, 2048]
    sinT = np.asarray(freqs_sin, dtype=np.float32).T
    # RoPE tables: partition p -> head p//64, hd dim d = p%64, pair j = d//2
    # rc[p] = cos[j], rs[p] = (-1 if d even else +1) * sin[j]
    j_of_p = (np.arange(128) % 64) // 2                  # [128]
    sign = np.where(np.arange(128) % 2 == 0, -1.0, 1.0).astype(np.float32)
    rc = np.ascontiguousarray(cosT[j_of_p]).astype(NP_MM_DT)
    rs = np.ascontiguousarray(sinT[j_of_p] * sign[:, None]).astype(NP_MM_DT)

    in_maps = []
    for c in range(NCORES):
        rows = slice(c * HLOC * HD, (c + 1) * HLOC * HD)   # 128 feat rows
        wq = w_qkv[0 * D : 1 * D][rows]                    # [128, 1024]
        wk = w_qkv[1 * D : 2 * D][rows]
        wv = w_qkv[2 * D : 3 * D][rows]
        wqkvT = np.ascontiguousarray(
            np.concatenate([wq, wk, wv], axis=0).T         # [1024, 384]
        ).astype(NP_MM_DT)
        woT = np.ascontiguousarray(w_o[:, rows].T).astype(NP_MM_DT)
        blob = np.concatenate(
            [a.reshape(-1) for a in (x, wqkvT, woT, rc, rs)]
        )
        in_maps.append({"blob": blob})
    return in_maps


_CACHE = {}


def _get_module():
    if "nc" in _CACHE:
        return _CACHE["nc"]
    nc = bacc.Bacc(
        "TRN2",
        target_bir_lowering=False,
        debug=False,
        enable_asserts=True,
        num_devices=NCORES,
    )
    # ONE packed input tensor: the axon/PJRT dispatch path costs ~100us
    # of host-side bookkeeping per argument array per dispatch (measured
    # with no-op jits), so 5 separate inputs would add ~500us to every
    # benchmarked execution.
    blob = nc.dram_tensor(
        "blob", [PACK_TOTAL], MM_DT, kind="ExternalInput"
    ).ap()

    def seg(off, size, pat, **axes):
        return blob[ds(off, size)].rearrange(pat, **axes)

    o_x, o_wqkv, o_wo, o_rc, o_rs, _ = PACK_OFFS
    ins = {
        "xT": seg(o_x, D * NT, "(a b) -> a b", a=D),
        "wqkvT": seg(o_wqkv, D * 384, "(a b) -> a b", a=D),
        "woT": seg(o_wo, 128 * D, "(a b) -> a b", a=128),
        "rc": seg(o_rc, 128 * L, "(a b) -> a b", a=128),
        "rs": seg(o_rs, 128 * L, "(a b) -> a b", a=128),
    }
    outs = {
        "out": nc.dram_tensor("out", [NT, D], MM_DT, kind="ExternalOutput").ap(),
    }
    with tile.TileContext(nc) as tc:
        build_body(tc, ins, outs)
    nc.compile()
    _CACHE["nc"] = nc
    return nc


def _get_runner():
    """Compiled SPMD runner with device-resident inputs (mirrors
    bass2jax.run_bass_via_pjrt, but caches the jitted callable and keeps
    inputs on device so repeat calls measure pure device execution)."""
    if "runner" in _CACHE:
        return _CACHE["runner"]
    import jax
    import jax.numpy as jnp
    from jax.experimental.shard_map import shard_map
    from jax.sharding import Mesh, NamedSharding, PartitionSpec

    from concourse import bass2jax, mybir as _mybir

    nc = _get_module()
    bass2jax.install_neuronx_cc_hook()

    part_name = nc.partition_id_tensor.name if nc.partition_id_tensor else None
    in_names, out_names, out_avals = [], [], []
    for alloc in nc.m.functions[0].allocations:
        if not isinstance(alloc, _mybir.MemoryLocationSet):
            continue
        name = alloc.memorylocations[0].name
        if alloc.kind == "ExternalInput":
            if name != part_name:
                in_names.append(name)
        elif alloc.kind == "ExternalOutput":
            shape = tuple(alloc.tensor_shape)
            dtype = _mybir.dt.np(alloc.dtype)
            out_names.append(name)
            out_avals.append(jax.core.ShapedArray(shape, dtype))
    n_params = len(in_names)
    # the output allocations are NOT ExternalInputs, so the lowering never
    # binds operands for them — outputs get fresh device buffers inside the
    # custom call. Passing zero-filled output operands (as the original
    # harness did) only adds dead arguments, and each argument array costs
    # ~100us of per-dispatch bookkeeping on the axon PJRT path.
    all_in_names = in_names
    if part_name is not None:
        all_in_names = all_in_names + [part_name]

    def _call(operands):
        if part_name is not None:
            operands = operands + [bass2jax.partition_id_tensor()]
        return tuple(
            bass2jax._bass_exec_p.bind(
                *operands,
                out_avals=tuple(out_avals),
                in_names=tuple(all_in_names),
                out_names=tuple(out_names),
                lowering_input_output_aliases=(),
                sim_require_finite=True,
                sim_require_nnan=True,
                nc=nc,
            )
        )

    def _body(*args):
        return _call(list(args))

    devices = jax.devices()[:NCORES]
    mesh = Mesh(np.asarray(devices), ("core",))
    spec = NamedSharding(mesh, PartitionSpec("core"))
    n_outs = len(out_avals)

    sharded = jax.jit(
        shard_map(
            _body,
            mesh=mesh,
            in_specs=(PartitionSpec("core"),) * n_params,
            out_specs=(PartitionSpec("core"),) * n_outs,
            check_rep=False,
        ),
        keep_unused=True,
    )

    zero_shapes = [(NCORES * a.shape[0], *a.shape[1:]) for a in out_avals]
    zeros_fn = jax.jit(
        lambda: tuple(
            jnp.zeros(s, a.dtype) for s, a in zip(zero_shapes, out_avals)
        ),
        out_shardings=(spec,) * n_outs,
    )

    runner = {
        "sharded": sharded,
        "zeros_fn": zeros_fn,
        "in_names": in_names,
        "out_names": out_names,
        "out_avals": out_avals,
        "spec": spec,
        "jax": jax,
    }
    _CACHE["runner"] = runner
    return runner


def _device_inputs(in_maps):
    r = _get_runner()
    jax = r["jax"]
    concat = [
        np.concatenate([in_maps[c][name] for c in range(NCORES)], axis=0)
        for name in r["in_names"]
    ]
    return [jax.device_put(a, r["spec"]) for a in concat]


def _run_once(dev_inputs):
    r = _get_runner()
    outs = r["sharded"](*dev_inputs)
    r["jax"].block_until_ready(outs)
    return outs


def bench(dev_inputs, iters=6, n_small=16, n_large=64):
    """Amortized per-execution device time. Single dispatches sit under an
    ~80-130 ms axon tunnel RTT with heavy jitter, so we pipeline N async
    dispatches, take the min total over several reps for each N (a stable
    floor statistic), and report the marginal cost between the two sizes."""
    import time as _time

    r = _get_runner()
    jax = r["jax"]

    def run_batch(n):
        t0 = _time.perf_counter()
        outs = [r["sharded"](*dev_inputs) for _ in range(n)]
        jax.block_until_ready(outs)
        return _time.perf_counter() - t0

    run_batch(1)  # warm
    t_small = min(run_batch(n_small) for _ in range(iters))
    t_large = min(run_batch(n_large) for _ in range(iters))
    est = (t_large - t_small) / (n_large - n_small)
    return max(est, 1e-9)


def kernel(hidden_states, w_qkv, w_o, freqs_cos, freqs_sin, mask=None):
    in_maps = _prep_inputs(hidden_states, w_qkv, w_o, freqs_cos, freqs_sin)
    dev_inputs = _device_inputs(in_maps)
    outs = _run_once(dev_inputs)
    out_g = np.asarray(outs[0]).reshape(NCORES, NT, D)
    acc = out_g.astype(np.float64).sum(axis=0)
    return acc.astype(np.float32).reshape(B, L, D)
